# revision 13
# baseline (speedup 1.0000x reference)
"""Trainium2 Bass kernel for 2-layer GAT (nn_GAT_34832184770812).

Strategy (8 NeuronCores, dst-node sharded):
- Each core owns 1250 dst nodes; node ids are rotated per core so own nodes
  are local rows 0:1250 (keeps the SPMD program identical across cores).
- Phase A: T1 = features @ [W1 | W1@al1 | W1@ar1] (bf16, replicated) ->
  DRAM gather table t1tab[N, 384] (feat 256 | el 256:260 | er 260:264 |
  pad; SBUF scratch ex at 264:268). Batched: 8 node tiles per superblock,
  2 loads + 1 store each.
- Phase B (layer-1 edge phase): edges sorted by dst window (128 dst rows
  per window, padded to 128-edge chunks, chunk counts uniform across
  cores). Per 1024-edge superchunk: ONE dma_gather of src rows; one-hot
  ind/indt tiles arrive in ONE batched DMA from a host-transposed
  [128, nchunk*2*128] layout; er per edge via indicator matmul
  (lhsT=ind); e = lrelu(el+er), ex = exp(e) (softmax shift-invariance
  makes max-subtraction unnecessary); messages scaled by ex; segment sum
  over dst via indicator-transpose matmul (lhsT=indt) with ex as extra
  rhs columns producing softmax denominators in the same psum.
- Window finalize: normalize, ELU, transpose (PE), T2own = h @ W2p,
  then AllGather THAT WINDOW's 128 t2 rows (collective overlaps the
  remaining edge phase).
- Phase D (layer-2 edge phase): same structure, 1 head, 47 feats;
  log_softmax batched after the last window to avoid Exp/Ln activation
  table thrash. Output [1250, 47] f32 per core, host concat.
"""

import numpy as np
import ml_dtypes

BF16 = ml_dtypes.bfloat16

# problem constants (hardcoded per contract)
N = 10000
E = 320000
IN_FEATS = 256
H = 4
D = 64
HD = 256
OUTF = 47
NEG = 0.2
NCORES = 8
OWN = N // NCORES          # 1250
P = 128
NWIN = (OWN + P - 1) // P  # 10 windows (last has 98 nodes)
WIN_SIZES = [min(P, OWN - P * w) for w in range(NWIN)]
K = 8                      # chunks per superchunk
ROW1 = 384                 # t1tab row (bf16): feat 256 | el 4 | er 4 | pad
C1 = 264                   # t1tab meaningful cols
EX1 = 264                  # SBUF scratch col for ex (within gathered tile)
ROW2 = 128                 # t2 row (bf16): feat 0:47 | el2 47 | er2 48 | ex2 49
T2OWN_ROWS = NWIN * P      # 1280 (rows 1250:1280 zeroed)
T2ALL_ROWS = NWIN * P * NCORES  # 10240

PER_WIN_AG = False         # per-window AllGather (overlapped) vs single AG

_CACHE = {}


# ----------------------------------------------------------------------------
# host-side graph preprocessing
# ----------------------------------------------------------------------------

def _prep_graph(src, dst):
    """Per-core edge partition, window sort, uniform padding, one-hots."""
    src = np.asarray(src).astype(np.int64)
    dst = np.asarray(dst).astype(np.int64)
    core_of = dst // OWN
    per_core = []
    for c in range(NCORES):
        sel = np.nonzero(core_of == c)[0]
        dl = dst[sel] - OWN * c                       # local dst in [0, OWN)
        sl = (src[sel] - OWN * c) % N                 # local src
        order = np.argsort(dl, kind="stable")
        per_core.append((dl[order], sl[order], src[sel][order]))

    # uniform chunks per window across cores
    cw = []
    bounds = []
    for c in range(NCORES):
        dl = per_core[c][0]
        b = np.searchsorted(dl, [P * w for w in range(NWIN + 1)])
        bounds.append(b)
    for w in range(NWIN):
        mx = max(bounds[c][w + 1] - bounds[c][w] for c in range(NCORES))
        cw.append((int(mx) + P - 1) // P)
    nchunk = sum(cw)
    nsc = (nchunk + K - 1) // K
    pad_chunks = nsc * K - nchunk
    cw[-1] += pad_chunks
    nchunk = nsc * K

    chunk_win = []
    for w in range(NWIN):
        chunk_win += [w] * cw[w]

    ES = nchunk * P
    out = []
    for c in range(NCORES):
        dl, sl, sg = per_core[c]
        b = bounds[c]
        src_loc = np.zeros(ES, np.int64)
        src_glb = np.zeros(ES, np.int64)
        dstrow = np.full(ES, -1, np.int64)            # -1 = dummy
        pos = 0
        for w in range(NWIN):
            e0, e1 = b[w], b[w + 1]
            n = e1 - e0
            src_loc[pos : pos + n] = sl[e0:e1]
            src_glb[pos : pos + n] = sg[e0:e1]
            dstrow[pos : pos + n] = dl[e0:e1] - P * w
            pos += cw[w] * P

        # layer-2 gather rows: global node n = core*1250 + l
        gc = src_glb // OWN
        gl = src_glb % OWN
        if PER_WIN_AG:
            # t2all row = w*1024 + core*128 + r
            t2row = (gl // P) * (P * NCORES) + gc * P + (gl % P)
        else:
            # t2all row = core*1280 + l
            t2row = gc * T2OWN_ROWS + gl

        # one-hot indicators, combined host-transposed layout
        # indm[p, ci, 0, e] = 1 iff edge e of chunk ci has dstrow p  (ind)
        # indm[p, ci, 1, d] = 1 iff edge p of chunk ci has dstrow d  (indt)
        indm = np.zeros((P, nchunk, 2, P), BF16)
        ch = np.arange(ES) // P
        e_in = np.arange(ES) % P
        real = dstrow >= 0
        r = np.nonzero(real)[0]
        indm[dstrow[r], ch[r], 0, e_in[r]] = 1
        indm[e_in[r], ch[r], 1, dstrow[r]] = 1

        # dma_gather wrapped idx layout, partition-major: [128, nsc*64]
        def wrap(ids):
            lay = np.zeros((P, nsc * (K * P // 16)), np.int16)
            for sc in range(nsc):
                blk = ids[sc * K * P : (sc + 1) * K * P].astype(np.int16)
                wr = np.zeros((16, K * P // 16), np.int16)
                kk = np.arange(K * P)
                wr[kk % 16, kk // 16] = blk
                lay[:, sc * (K * P // 16) : (sc + 1) * (K * P // 16)] = np.tile(wr, (8, 1))
            return lay

        out.append(dict(
            gidx1=wrap(src_loc),
            gidx2=wrap(t2row),
            indm=indm.reshape(P, nchunk * 2 * P),
        ))
    return out, cw, nchunk, nsc, chunk_win


# ----------------------------------------------------------------------------
# program build
# ----------------------------------------------------------------------------

def build_program(nchunk, nsc, chunk_win, reps=1, sim=False):
    import concourse.tile as tile
    from concourse import bacc, mybir
    from concourse.masks import make_identity

    NT = (N + P - 1) // P                              # 79 node tiles
    SB = (NT + K - 1) // K                             # 10 superblocks
    win_first = {}
    win_last = {}
    for ci, w in enumerate(chunk_win):
        if w not in win_first:
            win_first[w] = ci
        win_last[w] = ci

    nc = bacc.Bacc("TRN2", target_bir_lowering=False, debug=False, num_devices=NCORES,
                   dynamic_dma_scratch_size=32768)
    dt = mybir.dt
    featT = nc.declare_dram_parameter("featT", [IN_FEATS, N], dt.bfloat16, isOutput=False)
    W1p = nc.declare_dram_parameter("W1p", [IN_FEATS, C1], dt.bfloat16, isOutput=False)
    W2p = nc.declare_dram_parameter("W2p", [HD, 52], dt.bfloat16, isOutput=False)
    gidx1 = nc.declare_dram_parameter("gidx1", [P, nsc * (K * P // 16)], dt.int16, isOutput=False)
    gidx2 = nc.declare_dram_parameter("gidx2", [P, nsc * (K * P // 16)], dt.int16, isOutput=False)
    indmp = nc.declare_dram_parameter("indm", [P, nchunk * 2 * P], dt.bfloat16, isOutput=False)
    outp = nc.declare_dram_parameter("out", [OWN, OUTF], dt.float32, isOutput=True)

    t1tab = nc.dram_tensor("t1tab", [N, ROW1], dt.bfloat16)
    t2own = nc.dram_tensor("t2own", [T2OWN_ROWS, ROW2], dt.bfloat16)
    t2all = nc.dram_tensor("t2all", [T2ALL_ROWS, ROW2], dt.bfloat16)

    IW = K * P // 16                                   # 64 idx cols per sc
    IDXB = 4                                           # superchunks per idx load

    with tile.TileContext(nc) as tc:
        with (
            tc.tile_pool(name="const", bufs=1) as constp,
        ):
            ident = constp.tile([P, P], dt.float32)
            make_identity(nc, ident[:])
            zero52 = constp.tile([P, 52], dt.bfloat16)
            nc.vector.memset(zero52[:], 0)
            logits_sb = constp.tile([P, NWIN, OUTF], dt.float32)

            for r in range(reps):
                last = r == reps - 1
                # ---------------- phase A: T1 table ----------------
                with (
                    tc.tile_pool(name="pa", bufs=3) as pa,
                    tc.tile_pool(name="paps", bufs=4, space="PSUM") as paps,
                    tc.tile_pool(name="w1pool", bufs=1) as w1pool,
                ):
                    w1t = w1pool.tile([P, C1], dt.bfloat16, tag="w1a")
                    nc.sync.dma_start(out=w1t[:], in_=W1p[0:P, :])
                    w1b = w1pool.tile([P, C1], dt.bfloat16, tag="w1b")
                    nc.sync.dma_start(out=w1b[:], in_=W1p[P:IN_FEATS, :])
                    for sb in range(SB):
                        t0 = sb * K
                        nt_here = min(K, NT - t0)
                        m_all = min(K * P, N - t0 * P)
                        ft = pa.tile([P, 2, K * P], dt.bfloat16, tag="ft")
                        nc.sync.dma_start(out=ft[:, 0, 0:m_all], in_=featT[0:P, t0 * P : t0 * P + m_all])
                        nc.scalar.dma_start(out=ft[:, 1, 0:m_all], in_=featT[P : 2 * P, t0 * P : t0 * P + m_all])
                        row = pa.tile([P, K, C1], dt.bfloat16, tag="row")
                        for t in range(nt_here):
                            m = min(P, m_all - t * P)
                            ps = paps.tile([P, C1], dt.float32, space="PSUM", tag="paps")
                            nc.tensor.matmul(ps[0:m, :], lhsT=ft[:, 0, t * P : t * P + m], rhs=w1t[:], start=True, stop=False)
                            nc.tensor.matmul(ps[0:m, :], lhsT=ft[:, 1, t * P : t * P + m], rhs=w1b[:], start=False, stop=True)
                            nc.scalar.activation(row[0:m, t, :], ps[0:m, :], mybir.ActivationFunctionType.Copy)
                        nt_full = m_all // P
                        if nt_full:
                            nc.gpsimd.dma_start(
                                out=t1tab[t0 * P : t0 * P + nt_full * P, 0:C1].rearrange("(t p) c -> p t c", p=P),
                                in_=row[:, 0:nt_full, :],
                            )
                        m_r = m_all - nt_full * P
                        if m_r:
                            nc.gpsimd.dma_start(
                                out=t1tab[t0 * P + nt_full * P : t0 * P + m_all, 0:C1],
                                in_=row[0:m_r, nt_full, :],
                            )

                # zero t2own pad rows once
                nc.sync.dma_start(out=t2own[OWN:T2OWN_ROWS, 0:52], in_=zero52[0 : T2OWN_ROWS - OWN, :])

                # ---------------- phase B: layer-1 edge phase ----------------
                with (
                    tc.tile_pool(name="pb", bufs=4) as pb,
                    tc.tile_pool(name="pbi", bufs=5) as pbi,
                    tc.tile_pool(name="pbw", bufs=2) as pbw,
                    tc.tile_pool(name="pbfin", bufs=2) as pbfin,
                    tc.tile_pool(name="wps", bufs=2, space="PSUM") as wps,
                    tc.tile_pool(name="erps", bufs=4, space="PSUM") as erps,
                    tc.tile_pool(name="finps", bufs=2, space="PSUM") as finps,
                    tc.tile_pool(name="w2pool", bufs=1) as w2pool,
                ):
                    w2t = w2pool.tile([P, 52], dt.bfloat16, tag="w2a")
                    nc.sync.dma_start(out=w2t[:], in_=W2p[0:P, :])
                    w2b = w2pool.tile([P, 52], dt.bfloat16, tag="w2b")
                    nc.sync.dma_start(out=w2b[:], in_=W2p[P:HD, :])

                    erw_all = pbw.tile([P, NWIN, 4], dt.bfloat16, tag="erwall")
                    nc.sync.dma_start(
                        out=erw_all[:],
                        in_=t1tab[0 : NWIN * P, 260:264].rearrange("(w p) c -> p w c", p=P),
                    )
                    win_psum = None
                    idx_tiles = {}
                    indm_tiles = {}
                    g_tiles = {}
                    er_tiles = {}

                    def b_loads(s):
                        if s % IDXB == 0:
                            nb = min(IDXB, nsc - s)
                            t = pbi.tile([P, IDXB, IW], dt.int16, tag="idx")
                            nc.sync.dma_start(out=t[:, 0:nb, :], in_=gidx1[:, s * IW : (s + nb) * IW])
                            idx_tiles[s // IDXB] = t
                        t = pbi.tile([P, K, 2, P], dt.bfloat16, tag="indm")
                        nc.scalar.dma_start(out=t[:], in_=indmp[:, s * K * 2 * P : (s + 1) * K * 2 * P])
                        indm_tiles[s] = t

                    def b_gather(s):
                        g = pb.tile([P, K, ROW1], dt.bfloat16, tag="g")
                        nc.gpsimd.dma_gather(g[:], t1tab[:, :], idx_tiles[s // IDXB][:, s % IDXB, :], K * P, K * P, ROW1)
                        g_tiles[s] = g

                    def b_er(s):
                        ind_t = indm_tiles[s]
                        ep = erps.tile([P, K * 4], dt.float32, space="PSUM", tag="erp")
                        for j in range(K):
                            w = chunk_win[s * K + j]
                            nc.tensor.matmul(
                                ep[:, j * 4 : (j + 1) * 4],
                                lhsT=ind_t[:, j, 0, :], rhs=erw_all[:, w, :], start=True, stop=True,
                            )
                        er_tiles[s] = ep

                    for s in range(min(3, nsc)):
                        b_loads(s)
                    for s in range(min(2, nsc)):
                        b_gather(s)
                    b_er(0)

                    for sc in range(nsc):
                        if sc + 3 < nsc:
                            b_loads(sc + 3)
                        if sc + 2 < nsc:
                            b_gather(sc + 2)
                        g = g_tiles.pop(sc)
                        er_psum = er_tiles.pop(sc)
                        indm_t = indm_tiles.pop(sc)
                        att = pb.tile([P, K, 4], dt.float32, tag="att")
                        nc.vector.tensor_tensor(
                            out=att[:], in0=g[:, :, 256:260],
                            in1=er_psum[:].rearrange("p (c h) -> p c h", c=K),
                            op=mybir.AluOpType.add,
                        )
                        att2 = pb.tile([P, K, 4], dt.float32, tag="att2")
                        nc.vector.tensor_scalar_mul(att2[:], att[:], NEG)
                        nc.vector.tensor_tensor(out=att[:], in0=att[:], in1=att2[:], op=mybir.AluOpType.max)
                        nc.scalar.activation(g[:, :, EX1 : EX1 + 4], att[:], mybir.ActivationFunctionType.Exp)
                        nc.vector.tensor_tensor(
                            out=g[:, :, 0:HD].rearrange("p c (h d) -> p c h d", h=H),
                            in0=g[:, :, 0:HD].rearrange("p c (h d) -> p c h d", h=H),
                            in1=g[:, :, EX1 : EX1 + 4, None].broadcast_to([P, K, 4, D]),
                            op=mybir.AluOpType.mult,
                        )
                        if sc + 1 < nsc:
                            b_er(sc + 1)
                        for j in range(K):
                            ci = sc * K + j
                            w = chunk_win[ci]
                            if ci == win_first[w]:
                                win_psum = wps.tile([P, EX1 + 4], dt.float32, space="PSUM", tag="acc")
                            nc.tensor.matmul(
                                win_psum[:],
                                lhsT=indm_t[:, j, 1, :],
                                rhs=g[:, j, 0 : EX1 + 4],
                                start=(ci == win_first[w]),
                                stop=(ci == win_last[w]),
                            )
                            if ci == win_last[w]:
                                m = WIN_SIZES[w]
                                # normalize: h = msg / max(denom, eps)
                                den = pbfin.tile([P, 4], dt.float32, tag="den")
                                nc.vector.tensor_scalar_max(den[:], win_psum[:, EX1 : EX1 + 4], 1e-9)
                                rec = pbfin.tile([P, 4], dt.float32, tag="rec")
                                nc.vector.reciprocal(rec[:], den[:])
                                h_sb = pbfin.tile([P, HD], dt.float32, tag="hsb")
                                nc.vector.tensor_tensor(
                                    out=h_sb[:].rearrange("p (h d) -> p h d", h=H),
                                    in0=win_psum[:, 0:HD].rearrange("p (h d) -> p h d", h=H),
                                    in1=rec[:, :, None].broadcast_to([P, H, D]),
                                    op=mybir.AluOpType.mult,
                                )
                                # ELU: relu(h) + exp(min(h,0)) - 1
                                hneg = pbfin.tile([P, HD], dt.float32, tag="hneg")
                                nc.vector.tensor_scalar_min(hneg[:], h_sb[:], 0.0)
                                hexp = pbfin.tile([P, HD], dt.float32, tag="hexp")
                                nc.scalar.activation(hexp[:], hneg[:], mybir.ActivationFunctionType.Exp)
                                nc.vector.tensor_scalar_max(h_sb[:], h_sb[:], 0.0)
                                nc.vector.tensor_tensor(out=h_sb[:], in0=h_sb[:], in1=hexp[:], op=mybir.AluOpType.add)
                                nc.vector.tensor_scalar_add(h_sb[:], h_sb[:], -1.0)
                                # transpose h (2x PE) -> hT bf16
                                hT = pbfin.tile([P, 2, P], dt.bfloat16, tag="hT")
                                for half in range(2):
                                    tp = finps.tile([P, P], dt.float32, space="PSUM", tag="fin")
                                    nc.tensor.transpose(out=tp[:, 0:m], in_=h_sb[0:m, half * P : (half + 1) * P], identity=ident[0:m, 0:m])
                                    nc.vector.tensor_copy(hT[:, half, 0:m], tp[:, 0:m])
                                # T2own rows = h @ W2p
                                t2ps = finps.tile([P, 52], dt.float32, space="PSUM", tag="fin")
                                nc.tensor.matmul(t2ps[0:m, :], lhsT=hT[:, 0, 0:m], rhs=w2t[:], start=True, stop=False)
                                nc.tensor.matmul(t2ps[0:m, :], lhsT=hT[:, 1, 0:m], rhs=w2b[:], start=False, stop=True)
                                t2row = pbfin.tile([P, 52], dt.bfloat16, tag="t2row")
                                nc.vector.tensor_copy(t2row[0:m, :], t2ps[0:m, :])
                                nc.gpsimd.dma_start(out=t2own[w * P : w * P + m, 0:52], in_=t2row[0:m, :])
                                if PER_WIN_AG:
                                    # ---- phase C: per-window AllGather ----
                                    if sim:
                                        nc.gpsimd.dma_start(
                                            out=t2all[w * P * NCORES : w * P * NCORES + P, :],
                                            in_=t2own[w * P : (w + 1) * P, :],
                                        )
                                    else:
                                        nc.gpsimd.collective_compute(
                                            "AllGather",
                                            mybir.AluOpType.bypass,
                                            replica_groups=[list(range(NCORES))],
                                            ins=[t2own[w * P : (w + 1) * P, :]],
                                            outs=[t2all[w * P * NCORES : (w + 1) * P * NCORES, :]],
                                        )

                if not PER_WIN_AG:
                    # ---------------- phase C: single AllGather ----------------
                    if sim:
                        nc.gpsimd.dma_start(out=t2all[0:T2OWN_ROWS, :], in_=t2own[:, :])
                    else:
                        nc.gpsimd.collective_compute(
                            "AllGather",
                            mybir.AluOpType.bypass,
                            replica_groups=[list(range(NCORES))],
                            ins=[t2own[:, :]],
                            outs=[t2all[:, :]],
                        )

                # ---------------- phase D: layer-2 edge phase ----------------
                with (
                    tc.tile_pool(name="pd", bufs=4) as pd,
                    tc.tile_pool(name="pdi", bufs=5) as pdi,
                    tc.tile_pool(name="pdw", bufs=2) as pdw,
                    tc.tile_pool(name="pdfin", bufs=2) as pdfin,
                    tc.tile_pool(name="wps2", bufs=2, space="PSUM") as wps2,
                    tc.tile_pool(name="erps2", bufs=4, space="PSUM") as erps2,
                ):
                    erw2_all = pdw.tile([P, NWIN], dt.bfloat16, tag="erw2all")
                    nc.sync.dma_start(
                        out=erw2_all[:],
                        in_=t2own[0 : NWIN * P, 48:49].rearrange("(w p) c -> p (w c)", p=P),
                    )
                    win_psum2 = None
                    idx_tiles2 = {}
                    indm_tiles2 = {}
                    g_tiles2 = {}
                    er_tiles2 = {}

                    def d_loads(s):
                        if s % IDXB == 0:
                            nb = min(IDXB, nsc - s)
                            t = pdi.tile([P, IDXB, IW], dt.int16, tag="idx2")
                            nc.sync.dma_start(out=t[:, 0:nb, :], in_=gidx2[:, s * IW : (s + nb) * IW])
                            idx_tiles2[s // IDXB] = t
                        t = pdi.tile([P, K, 2, P], dt.bfloat16, tag="indm2")
                        nc.scalar.dma_start(out=t[:], in_=indmp[:, s * K * 2 * P : (s + 1) * K * 2 * P])
                        indm_tiles2[s] = t

                    def d_gather(s):
                        g2 = pd.tile([P, K, ROW2], dt.bfloat16, tag="g2")
                        nc.gpsimd.dma_gather(g2[:], t2all[:, :], idx_tiles2[s // IDXB][:, s % IDXB, :], K * P, K * P, ROW2)
                        g_tiles2[s] = g2

                    def d_er(s):
                        ind_t = indm_tiles2[s]
                        ep = erps2.tile([P, K], dt.float32, space="PSUM", tag="erp2")
                        for j in range(K):
                            w = chunk_win[s * K + j]
                            nc.tensor.matmul(
                                ep[:, j : j + 1],
                                lhsT=ind_t[:, j, 0, :], rhs=erw2_all[:, w : w + 1], start=True, stop=True,
                            )
                        er_tiles2[s] = ep

                    for s in range(min(3, nsc)):
                        d_loads(s)
                    for s in range(min(2, nsc)):
                        d_gather(s)
                    d_er(0)

                    for sc in range(nsc):
                        if sc + 3 < nsc:
                            d_loads(sc + 3)
                        if sc + 2 < nsc:
                            d_gather(sc + 2)
                        g2 = g_tiles2.pop(sc)
                        er_psum2 = er_tiles2.pop(sc)
                        indm_t = indm_tiles2.pop(sc)
                        att = pd.tile([P, K], dt.float32, tag="attl2")
                        nc.vector.tensor_tensor(
                            out=att[:, :, None], in0=g2[:, :, 47:48], in1=er_psum2[:, :, None],
                            op=mybir.AluOpType.add,
                        )
                        att2 = pd.tile([P, K], dt.float32, tag="attl2b")
                        nc.vector.tensor_scalar_mul(att2[:], att[:], NEG)
                        nc.vector.tensor_tensor(out=att[:], in0=att[:], in1=att2[:], op=mybir.AluOpType.max)
                        nc.scalar.activation(g2[:, :, 49:50], att[:, :, None], mybir.ActivationFunctionType.Exp)
                        nc.vector.tensor_tensor(
                            out=g2[:, :, 0:48],
                            in0=g2[:, :, 0:48],
                            in1=g2[:, :, 49:50].broadcast_to([P, K, 48]),
                            op=mybir.AluOpType.mult,
                        )
                        if sc + 1 < nsc:
                            d_er(sc + 1)
                        for j in range(K):
                            ci = sc * K + j
                            w = chunk_win[ci]
                            if ci == win_first[w]:
                                win_psum2 = wps2.tile([P, 50], dt.float32, space="PSUM", tag="acc2")
                            nc.tensor.matmul(
                                win_psum2[:],
                                lhsT=indm_t[:, j, 1, :],
                                rhs=g2[:, j, 0:50],
                                start=(ci == win_first[w]),
                                stop=(ci == win_last[w]),
                            )
                            if ci == win_last[w]:
                                den = pdfin.tile([P, 1], dt.float32, tag="den2")
                                nc.vector.tensor_scalar_max(den[:], win_psum2[:, 49:50], 1e-9)
                                rec = pdfin.tile([P, 1], dt.float32, tag="rec2")
                                nc.vector.reciprocal(rec[:], den[:])
                                nc.vector.tensor_scalar(
                                    out=logits_sb[:, w, :], in0=win_psum2[:, 0:OUTF],
                                    scalar1=rec[:, 0:1], scalar2=None,
                                    op0=mybir.AluOpType.mult,
                                )
                    # batched log_softmax over all windows (one Ln table load)
                    mx = pdfin.tile([P, NWIN], dt.float32, tag="mx")
                    nc.vector.tensor_reduce(mx[:], logits_sb[:], mybir.AxisListType.X, mybir.AluOpType.max)
                    nc.vector.tensor_tensor(
                        out=logits_sb[:], in0=logits_sb[:],
                        in1=mx[:, :, None].broadcast_to([P, NWIN, OUTF]),
                        op=mybir.AluOpType.subtract,
                    )
                    se = pdfin.tile([P, NWIN], dt.float32, tag="se")
                    exps = pdfin.tile([P, OUTF], dt.float32, tag="exps")
                    for w in range(NWIN):
                        nc.scalar.activation(exps[:], logits_sb[:, w, :], mybir.ActivationFunctionType.Exp, accum_out=se[:, w : w + 1])
                    lse = pdfin.tile([P, NWIN], dt.float32, tag="lse")
                    nc.scalar.activation(lse[:], se[:], mybir.ActivationFunctionType.Ln)
                    nc.vector.tensor_tensor(
                        out=logits_sb[:], in0=logits_sb[:],
                        in1=lse[:, :, None].broadcast_to([P, NWIN, OUTF]),
                        op=mybir.AluOpType.subtract,
                    )
                    if last:
                        for w in range(NWIN):
                            m = WIN_SIZES[w]
                            nc.gpsimd.dma_start(out=outp[w * P : w * P + m, :], in_=logits_sb[0:m, w, :])
    nc.compile()
    return nc


# ----------------------------------------------------------------------------
# host entry
# ----------------------------------------------------------------------------

def _host_inputs(features, src, dst, W1, al1, ar1, W2, al2, ar2):
    feats = np.asarray(features, np.float32)
    W1 = np.asarray(W1, np.float32)
    W2 = np.asarray(W2, np.float32)
    al1 = np.asarray(al1, np.float32)
    ar1 = np.asarray(ar1, np.float32)
    al2 = np.asarray(al2, np.float32)
    ar2 = np.asarray(ar2, np.float32)

    Wl1 = np.stack([W1[:, h * D : (h + 1) * D] @ al1[h] for h in range(H)], axis=1)
    Wr1 = np.stack([W1[:, h * D : (h + 1) * D] @ ar1[h] for h in range(H)], axis=1)
    W1p = np.concatenate([W1, Wl1, Wr1], axis=1).astype(BF16)          # [256, 264]
    Wl2 = (W2 @ al2[0])[:, None]
    Wr2 = (W2 @ ar2[0])[:, None]
    W2p = np.concatenate([W2, Wl2, Wr2, np.zeros((HD, 3), np.float32)], axis=1).astype(BF16)  # [256, 52]

    graph, cw, nchunk, nsc, chunk_win = _prep_graph(src, dst)
    featT = np.ascontiguousarray(feats.T)                               # [256, N]
    in_maps = []
    for c in range(NCORES):
        featTl = np.roll(featT, -OWN * c, axis=1)                       # local node order
        in_maps.append(dict(
            featT=featTl.astype(BF16),
            W1p=W1p, W2p=W2p,
            gidx1=graph[c]["gidx1"], gidx2=graph[c]["gidx2"],
            indm=graph[c]["indm"],
        ))
    return in_maps, nchunk, nsc, chunk_win


def kernel(features, src, dst, W1, al1, ar1, W2, al2, ar2):
    from concourse.bass_utils import run_bass_kernel_spmd

    in_maps, nchunk, nsc, chunk_win = _host_inputs(
        features, src, dst, W1, al1, ar1, W2, al2, ar2)
    key = (nchunk, nsc, tuple(chunk_win))
    if key not in _CACHE:
        _CACHE[key] = build_program(nchunk, nsc, chunk_win, reps=1)
    nc = _CACHE[key]
    res = run_bass_kernel_spmd(nc, in_maps, core_ids=list(range(NCORES)))
    return np.concatenate([res.results[c]["out"] for c in range(NCORES)], axis=0)
